# revision 1
# baseline (speedup 1.0000x reference)
"""2D Gaussian splat rasterizer on 8 Trainium2 NeuronCores.

Strategy: shard the image into 8 row-bands (one per core). Each band is
tiled into (8-row strip) x (128-col half) tiles. Per tile, gaussians are
culled host-side by their raster_ratio-sigma bounding box and packed into
chunks of 128 (partition dim). On device, per (tile, chunk):

    arg   = coefT.T @ basis        TensorE, K=6 fp32: -0.5*mahal2 in the
                                   6-term pixel basis [x^2, xy, y^2, x, y, 1]
                                   (tile-local coords for fp32 accuracy)
    w     = Exp(arg + ln(opacity)) ScalarE, per-partition bias, PSUM src
    alpha = (arg >= -r^2/2) * w    VectorE scalar_tensor_tensor, fp16 out
    out  += colors.T @ alpha       TensorE, K=128 fp16, PSUM accumulate

The [3, F] accumulator is copied out per tile and the full [H, W, 3]
image is reassembled host-side (pure concatenation; no collectives).
"""

import numpy as np
import concourse.bacc as bacc
import concourse.tile as tile
from concourse import mybir
from concourse.bass_utils import run_bass_kernel_spmd

_runner_cache = {}


def _get_runner(nc):
    """Persistent jitted SPMD executor for a compiled Bass program (modeled on
    bass2jax.run_bass_via_pjrt's multi-core path, but cached so repeat calls
    reuse the same XLA executable — no retrace, no NEFF reload)."""
    key = id(nc)
    if key in _runner_cache:
        return _runner_cache[key]
    import jax
    import jax.numpy as jnp
    from jax.sharding import Mesh, PartitionSpec
    from jax.experimental.shard_map import shard_map
    from concourse import bass2jax, mybir as mb

    bass2jax.install_neuronx_cc_hook()

    in_names, out_names, out_avals, zero_outs = [], [], [], []
    partition_name = nc.partition_id_tensor.name if nc.partition_id_tensor else None
    for alloc in nc.m.functions[0].allocations:
        if not isinstance(alloc, mb.MemoryLocationSet):
            continue
        name = alloc.memorylocations[0].name
        if alloc.kind == "ExternalInput":
            if name != partition_name:
                in_names.append(name)
        elif alloc.kind == "ExternalOutput":
            shape = tuple(alloc.tensor_shape)
            dtype = mb.dt.np(alloc.dtype)
            out_names.append(name)
            out_avals.append(jax.core.ShapedArray(shape, dtype))
            zero_outs.append(np.zeros(shape, dtype))
    n_params = len(in_names)
    all_in = in_names + out_names + ([partition_name] if partition_name else [])

    def _body(*args):
        operands = list(args)
        if partition_name is not None:
            operands.append(bass2jax.partition_id_tensor())
        outs = bass2jax._bass_exec_p.bind(
            *operands,
            out_avals=tuple(out_avals),
            in_names=tuple(all_in),
            out_names=tuple(out_names),
            lowering_input_output_aliases=(),
            sim_require_finite=True,
            sim_require_nnan=True,
            nc=nc,
        )
        return tuple(outs)

    devices = jax.devices()[:N_CORES]
    mesh = Mesh(np.asarray(devices), ("core",))
    in_specs = (PartitionSpec("core"),) * (n_params + len(out_names))
    out_specs = (PartitionSpec("core"),) * len(out_names)
    sharded = jax.jit(
        shard_map(
            _body, mesh=mesh, in_specs=in_specs, out_specs=out_specs, check_rep=False
        ),
        donate_argnums=tuple(range(n_params, n_params + len(out_names))),
        keep_unused=True,
    )

    dev_in_cache = {}

    def run(in_maps, reuse_inputs=False):
        if reuse_inputs and "in" in dev_in_cache:
            concat_in = dev_in_cache["in"]
        else:
            concat_in = [
                np.concatenate([np.asarray(m[nm]) for m in in_maps], axis=0)
                for nm in in_names
            ]
            if reuse_inputs:
                from jax.sharding import NamedSharding

                sh = NamedSharding(mesh, PartitionSpec("core"))
                concat_in = [jax.device_put(a, sh) for a in concat_in]
                for a in concat_in:
                    a.block_until_ready()
                dev_in_cache["in"] = concat_in
        concat_zeros = [
            np.zeros((N_CORES * z.shape[0], *z.shape[1:]), z.dtype) for z in zero_outs
        ]
        out_arrs = sharded(*concat_in, *concat_zeros)
        out_arrs = [a.block_until_ready() for a in out_arrs]
        return [
            {
                nm: np.asarray(out_arrs[i]).reshape(N_CORES, *out_avals[i].shape)[c]
                for i, nm in enumerate(out_names)
            }
            for c in range(N_CORES)
        ]

    def time_loop(in_maps, n_calls):
        """Per-call wall times with inputs and donated zero-outputs pre-staged
        on device; outputs stay on device (only block_until_ready)."""
        import time as _t
        from jax.sharding import NamedSharding

        sh = NamedSharding(mesh, PartitionSpec("core"))
        concat_in = [
            jax.device_put(
                np.concatenate([np.asarray(m[nm]) for m in in_maps], axis=0), sh
            )
            for nm in in_names
        ]
        zeros_sets = [
            [
                jax.device_put(
                    np.zeros((N_CORES * z.shape[0], *z.shape[1:]), z.dtype), sh
                )
                for z in zero_outs
            ]
            for _ in range(n_calls)
        ]
        for a in concat_in:
            a.block_until_ready()
        for zs in zeros_sets:
            for a in zs:
                a.block_until_ready()
        # warm once (executable load)
        outs = sharded(*concat_in, *zeros_sets[0])
        [a.block_until_ready() for a in outs]
        times = []
        for i in range(1, n_calls):
            t0 = _t.perf_counter()
            outs = sharded(*concat_in, *zeros_sets[i])
            [a.block_until_ready() for a in outs]
            times.append(_t.perf_counter() - t0)
        return times

    def stage(in_maps, n_calls):
        """Pre-stage inputs + n_calls sets of donated zeros; return a closure
        that executes once per call (device exec + block)."""
        from jax.sharding import NamedSharding

        sh = NamedSharding(mesh, PartitionSpec("core"))
        concat_in = [
            jax.device_put(
                np.concatenate([np.asarray(m[nm]) for m in in_maps], axis=0), sh
            )
            for nm in in_names
        ]
        zeros_sets = [
            [
                jax.device_put(
                    np.zeros((N_CORES * z.shape[0], *z.shape[1:]), z.dtype), sh
                )
                for z in zero_outs
            ]
            for _ in range(n_calls)
        ]
        for a in concat_in:
            a.block_until_ready()
        for zs in zeros_sets:
            for a in zs:
                a.block_until_ready()
        state = {"i": 0}

        def call():
            i = state["i"]
            state["i"] += 1
            outs = sharded(*concat_in, *zeros_sets[i])
            # force full materialization — under the axon proxy,
            # block_until_ready alone does not wait for device execution
            return [np.asarray(a) for a in outs]

        return call

    def stage_async(in_maps, n_calls):
        """Like stage() but returns call(block=False) that does not wait."""
        from jax.sharding import NamedSharding

        sh = NamedSharding(mesh, PartitionSpec("core"))
        concat_in = [
            jax.device_put(
                np.concatenate([np.asarray(m[nm]) for m in in_maps], axis=0), sh
            )
            for nm in in_names
        ]
        zeros_sets = [
            [
                jax.device_put(
                    np.zeros((N_CORES * z.shape[0], *z.shape[1:]), z.dtype), sh
                )
                for z in zero_outs
            ]
            for _ in range(n_calls)
        ]
        for a in concat_in:
            a.block_until_ready()
        for zs in zeros_sets:
            for a in zs:
                a.block_until_ready()
        state = {"i": 0}

        def call(block=False):
            i = state["i"]
            state["i"] += 1
            outs = sharded(*concat_in, *zeros_sets[i])
            if block:
                outs = [np.asarray(a) for a in outs]
            return outs

        return call

    run.time_loop = time_loop
    run.stage = stage
    run.stage_async = stage_async
    _runner_cache[key] = run
    return run

N_CORES = 8
K = 6
STRIP_ROWS = 16
TILE_COLS = 64
F = STRIP_ROWS * TILE_COLS  # pixels per tile

_prog_cache = {}


def _build_program(slot_nch, cutoff, repeat=1):
    """One SPMD program: per tile-slot s, slot_nch[s] chunks of 128 gaussians.

    The two fp32 K=6 arg matmuls per chunk are row-tiled into PE row-groups
    0 and 1 (tile_position), so they run concurrently in different 32-row
    strips of the array. The basis/coef SBUF images carry the operands at
    base partitions 0 and 32 (host replicates the coefs)."""
    n_slots = len(slot_nch)
    tot = sum(slot_nch)
    nc = bacc.Bacc(
        "TRN2",
        target_bir_lowering=False,
        debug=False,
        enable_asserts=True,
        num_devices=N_CORES,
    )
    f32, f16 = mybir.dt.float32, mybir.dt.float16
    coef_ext = nc.dram_tensor("coef", [102, tot * 128], f32, kind="ExternalInput").ap()
    basis_ext = nc.dram_tensor("basis", [102, F // 2], f32, kind="ExternalInput").ap()
    lnop_ext = nc.dram_tensor("lnop", [128, tot], f32, kind="ExternalInput").ap()
    colors_ext = nc.dram_tensor("colors", [128, tot * 3], f16, kind="ExternalInput").ap()
    out_ext = nc.dram_tensor("out", [n_slots * 6, F // 2], f32, kind="ExternalOutput").ap()

    with tile.TileContext(nc) as tc:
        with (
            tc.tile_pool(name="consts", bufs=1) as consts,
            tc.tile_pool(name="work", bufs=3) as work,
            tc.tile_pool(name="outsb", bufs=2) as outsb,
            tc.tile_pool(name="psum", bufs=3, space="PSUM") as psum,
            tc.tile_pool(name="psum_out", bufs=2, space="PSUM") as psum_out,
        ):
            basis_sb = consts.tile([102, F // 2], f32)
            nc.sync.dma_start(out=basis_sb[:], in_=basis_ext[:])
            coef_sb = consts.tile([102, tot * 128], f32)
            nc.sync.dma_start(out=coef_sb[:], in_=coef_ext[:])
            lnop_sb = consts.tile([128, tot], f32)
            nc.sync.dma_start(out=lnop_sb[:], in_=lnop_ext[:])
            colors_sb = consts.tile([128, tot * 3], f16)
            nc.sync.dma_start(out=colors_sb[:], in_=colors_ext[:])

            base = 0
            for s, n in enumerate(slot_nch):
                # [35, F/2]: rows 0-2 <- pixel cols 0:F/2 (col-group 0),
                # rows 32-34 <- pixel cols F/2:F (col-group 1)
                out_ps = psum_out.tile([35, F // 2], f32, tag="out")
                for rep in range(repeat):
                    for c in range(n):
                        j = base + c
                        arg_ps = psum.tile([128, F], f32, tag="arg")
                        for gi, h in enumerate(range(0, F, 512)):
                            p0 = 32 * gi + 64 * ((base + c) % 2)
                            nc.tensor.matmul(
                                arg_ps[:, h : h + 512],
                                lhsT=coef_sb[p0 : p0 + K, j * 128 : (j + 1) * 128],
                                rhs=basis_sb[p0 : p0 + K, :],
                                start=True,
                                stop=True,
                                tile_position=(p0, 0),
                            )
                        w_sb = work.tile([128, F], f16, tag="w")
                        nc.scalar.activation(
                            w_sb[:],
                            arg_ps[:],
                            mybir.ActivationFunctionType.Exp,
                            bias=lnop_sb[:, j : j + 1],
                            scale=1.0,
                        )
                        alpha_sb = work.tile([128, F], f16, tag="alpha")
                        nc.vector.scalar_tensor_tensor(
                            out=alpha_sb[:],
                            in0=arg_ps[:],
                            scalar=float(cutoff),
                            in1=w_sb[:],
                            op0=mybir.AluOpType.is_ge,
                            op1=mybir.AluOpType.mult,
                        )
                        for gi in range(2):
                            p0 = 32 * gi
                            nc.tensor.matmul(
                                out_ps[p0 : p0 + 3, :],
                                lhsT=colors_sb[:, j * 3 : (j + 1) * 3],
                                rhs=alpha_sb[:, gi * (F // 2) : (gi + 1) * (F // 2)],
                                start=(c == 0 and rep == 0),
                                stop=(c == n - 1 and rep == repeat - 1),
                                tile_position=(0, p0),
                            )
                out_sb = outsb.tile([35, F // 2], f32, tag="osb")
                if s % 2 == 0:
                    nc.scalar.copy(out_sb[:], out_ps[:])
                else:
                    nc.vector.tensor_copy(out_sb[:], out_ps[:])
                nc.sync.dma_start(
                    out=out_ext[s * 6 : s * 6 + 3, :], in_=out_sb[0:3, :]
                )
                nc.sync.dma_start(
                    out=out_ext[s * 6 + 3 : s * 6 + 6, :], in_=out_sb[32:35, :]
                )
                base += n
    nc.compile()
    return nc


def _get_program(slot_nch, cutoff, repeat=1):
    key = (tuple(slot_nch), float(cutoff), repeat)
    if key not in _prog_cache:
        _prog_cache[key] = _build_program(slot_nch, cutoff, repeat)
    return _prog_cache[key]


def _coefs(means, stds, rhos, cxo, cyo):
    """[6, G] coefficients of -0.5*mahal2 in local coords; f64 intermediates."""
    sx = stds[:, 0].astype(np.float64)
    sy = stds[:, 1].astype(np.float64)
    r = rhos.astype(np.float64)
    om = 1.0 - r * r
    ia = 1.0 / (sx * sx * om)
    ib = -r / (sx * sy * om)
    ic = 1.0 / (sy * sy * om)
    mxl = means[:, 0].astype(np.float64) - cxo
    myl = means[:, 1].astype(np.float64) - cyo
    return np.stack(
        [
            -0.5 * ia,
            -ib,
            -0.5 * ic,
            ia * mxl + ib * myl,
            ib * mxl + ic * myl,
            -0.5 * (ia * mxl * mxl + 2 * ib * mxl * myl + ic * myl * myl),
        ],
        axis=0,
    ).astype(np.float32)


def _basis(cxo_off=TILE_COLS / 2, cyo_off=STRIP_ROWS / 2):
    ys = np.arange(STRIP_ROWS, dtype=np.float64) + 0.5 - cyo_off
    xs = np.arange(TILE_COLS, dtype=np.float64) + 0.5 - cxo_off
    yl = np.repeat(ys, TILE_COLS)
    xl = np.tile(xs, STRIP_ROWS)
    return np.stack(
        [xl * xl, xl * yl, yl * yl, xl, yl, np.ones_like(xl)], axis=0
    ).astype(np.float32)


def kernel(
    opacity,
    means,
    stds,
    rhos,
    colors,
    image_height,
    image_width,
    scale_factor,
    raster_ratio,
    _repeat=1,
    _time_exec=False,
    _bench_calls=0,
):
    H = int(image_height)
    W = int(image_width)
    sf = float(scale_factor)
    rr = float(raster_ratio)
    opacity = np.asarray(opacity, np.float32)
    means = np.asarray(means, np.float32)
    stds = np.asarray(stds, np.float32) * np.float32(sf)
    rhos = np.asarray(rhos, np.float32)
    colors = np.asarray(colors, np.float32)
    N = opacity.shape[0]

    n_tiles_y = H // STRIP_ROWS
    n_tiles_x = W // TILE_COLS
    n_tiles = n_tiles_y * n_tiles_x
    assert n_tiles % N_CORES == 0
    n_slots = n_tiles // N_CORES
    cutoff = -0.5 * rr * rr

    # --- host-side cull: bbox of the rr-sigma ellipse vs tile pixel centers
    ex = rr * stds[:, 0].astype(np.float64) + 0.01
    ey = rr * stds[:, 1].astype(np.float64) + 0.01
    mx = means[:, 0].astype(np.float64)
    my = means[:, 1].astype(np.float64)

    tile_ids = []  # per tile: gaussian index array
    tile_pos = []  # per tile: (ty, tx) pixel origin
    for tyi in range(n_tiles_y):
        ty = tyi * STRIP_ROWS
        ymask = (my + ey >= ty + 0.5) & (my - ey <= ty + STRIP_ROWS - 0.5)
        for txi in range(n_tiles_x):
            tx = txi * TILE_COLS
            m = ymask & (mx + ex >= tx + 0.5) & (mx - ex <= tx + TILE_COLS - 0.5)
            tile_ids.append(np.nonzero(m)[0])
            tile_pos.append((ty, tx))

    # snake-deal tiles to cores by descending chunk need, so every core gets a
    # near-identical sorted chunk profile (SPMD: slot capacity is the max
    # over cores at each slot position)
    nchs = [max(1, (len(ids) + 127) // 128) for ids in tile_ids]
    t_order = sorted(range(n_tiles), key=lambda t: -nchs[t])
    assign = [[] for _ in range(N_CORES)]
    for i, t in enumerate(t_order):
        rnd, pos = divmod(i, N_CORES)
        core = pos if rnd % 2 == 0 else N_CORES - 1 - pos
        assign[core].append(t)
    slot_nch = tuple(
        max(nchs[assign[core][k]] for core in range(N_CORES)) for k in range(n_slots)
    )
    tot = sum(slot_nch)

    nc = _get_program(slot_nch, cutoff, _repeat)

    basis6 = _basis()  # [6, F]
    basis = np.zeros((102, F // 2), np.float32)
    for p0, half in ((0, 0), (32, 1), (64, 0), (96, 1)):
        basis[p0 : p0 + K] = basis6[:, half * (F // 2) : (half + 1) * (F // 2)]
    lnop_all = np.where(
        opacity > 0, np.log(np.maximum(opacity, 1e-45)), -1e4
    ).astype(np.float32)

    in_maps = []
    perms = []  # per core: slot -> (ty, tx)
    for core in range(N_CORES):
        coef_arr = np.zeros((102, tot * 128), np.float32)
        lnop_arr = np.full((128, tot), -1e4, np.float32)
        colors_arr = np.zeros((128, tot * 3), np.float16)
        perm = []
        base = 0
        for k in range(n_slots):
            t = assign[core][k]
            ty, tx = tile_pos[t]
            perm.append((ty, tx))
            ids = tile_ids[t]
            g = len(ids)
            assert g <= slot_nch[k] * 128
            if g:
                cxo = tx + TILE_COLS / 2
                cyo = ty + STRIP_ROWS / 2
                cf = _coefs(means[ids], stds[ids], rhos[ids], cxo, cyo)
                for p0 in (0, 32, 64, 96):
                    coef_arr[p0 : p0 + K, base * 128 : base * 128 + g] = cf
                ln = lnop_all[ids]
                col = colors[ids].astype(np.float16)
                # scatter into [128, nch] column-major-by-chunk layout
                for c in range((g + 127) // 128):
                    lo, hi = c * 128, min((c + 1) * 128, g)
                    lnop_arr[: hi - lo, base + c] = ln[lo:hi]
                    colors_arr[: hi - lo, (base + c) * 3 : (base + c) * 3 + 3] = col[
                        lo:hi
                    ]
            base += slot_nch[k]
        perms.append(perm)
        in_maps.append(
            {
                "coef": coef_arr,
                "basis": basis,
                "lnop": lnop_arr,
                "colors": colors_arr,
            }
        )

    import time as _time

    global _last_in_maps
    _last_in_maps = in_maps
    run = _get_runner(nc)
    if _bench_calls:
        return run.time_loop(in_maps, _bench_calls)
    t0 = _time.time()
    results = run(in_maps, reuse_inputs=_time_exec)
    exec_wall = _time.time() - t0

    out = np.zeros((H, W, 3), np.float32)
    hh = STRIP_ROWS // 2
    for core in range(N_CORES):
        o = results[core]["out"]  # [n_slots*6, F/2]: per slot 2 half-tiles
        for k, (ty, tx) in enumerate(perms[core]):
            for gi in range(2):
                blk = o[k * 6 + gi * 3 : k * 6 + gi * 3 + 3, :].reshape(
                    3, hh, TILE_COLS
                )
                y = ty + gi * hh
                out[y : y + hh, tx : tx + TILE_COLS, :] = blk.transpose(1, 2, 0)
    if _repeat > 1:
        out /= np.float32(_repeat)
    if _time_exec:
        return out, exec_wall
    return out



# revision 9
# speedup vs baseline: 2.8586x; 2.8586x over previous
"""2D Gaussian splat rasterizer on 8 Trainium2 NeuronCores.

Strategy: shard the image into 128 tiles of 16x32 px (F=512), dealt to 8
cores (16 slots each). Gaussians are culled host-side per tile by the
EXACT min-Mahalanobis-over-rect test (<= raster_ratio^2) and packed into
chunks of <=128 (partition dim). Per chunk, on device:

    arg   = coefT.T @ basis      TensorE, K=12 fp16, 1 cy/row: the 6-term
                                 pixel-basis [x2, xy, y2, x, y, 1] quadratic
                                 (tile-local coords) with each coefficient
                                 split hi/lo across two fp16 rows (full-f32
                                 effective precision; basis values are all
                                 exactly representable in fp16), constant
                                 row folds in ln(opacity)
    alpha = Exp(arg)             ScalarE from PSUM, fp16 out, fused over
                                 FUSE consecutive chunks (no bias, no mask:
                                 the cutoff is applied by the exact cull,
                                 in-tile tails are within tolerance)
    out  += colors.T @ alpha     TensorE, K=128 fp16, PSUM accumulate;
                                 4 tiles share one PSUM bank via
                                 tile_position column groups {0,32,64,96}

Per round of 4 tiles one DVE copy moves the PSUM bank to fp16 SBUF; one
final DMA (partition-strided AP) writes all 16 tiles out. 3 DMAs total.
The full [H, W, 3] image is reassembled host-side (no collectives).
"""

import numpy as np
import concourse.bacc as bacc
import concourse.tile as tile
from concourse import mybir
from concourse.bass_utils import run_bass_kernel_spmd

_runner_cache = {}


def _get_runner(nc):
    """Persistent jitted SPMD executor for a compiled Bass program (modeled on
    bass2jax.run_bass_via_pjrt's multi-core path, but cached so repeat calls
    reuse the same XLA executable — no retrace, no NEFF reload)."""
    key = id(nc)
    if key in _runner_cache:
        return _runner_cache[key]
    import jax
    import jax.numpy as jnp
    from jax.sharding import Mesh, PartitionSpec
    from jax.experimental.shard_map import shard_map
    from concourse import bass2jax, mybir as mb

    bass2jax.install_neuronx_cc_hook()

    in_names, out_names, out_avals, zero_outs = [], [], [], []
    partition_name = nc.partition_id_tensor.name if nc.partition_id_tensor else None
    for alloc in nc.m.functions[0].allocations:
        if not isinstance(alloc, mb.MemoryLocationSet):
            continue
        name = alloc.memorylocations[0].name
        if alloc.kind == "ExternalInput":
            if name != partition_name:
                in_names.append(name)
        elif alloc.kind == "ExternalOutput":
            shape = tuple(alloc.tensor_shape)
            dtype = mb.dt.np(alloc.dtype)
            out_names.append(name)
            out_avals.append(jax.core.ShapedArray(shape, dtype))
            zero_outs.append(np.zeros(shape, dtype))
    n_params = len(in_names)
    all_in = in_names + out_names + ([partition_name] if partition_name else [])

    def _body(*args):
        operands = list(args)
        if partition_name is not None:
            operands.append(bass2jax.partition_id_tensor())
        outs = bass2jax._bass_exec_p.bind(
            *operands,
            out_avals=tuple(out_avals),
            in_names=tuple(all_in),
            out_names=tuple(out_names),
            lowering_input_output_aliases=(),
            sim_require_finite=True,
            sim_require_nnan=True,
            nc=nc,
        )
        return tuple(outs)

    devices = jax.devices()[:N_CORES]
    mesh = Mesh(np.asarray(devices), ("core",))
    in_specs = (PartitionSpec("core"),) * (n_params + len(out_names))
    out_specs = (PartitionSpec("core"),) * len(out_names)
    sharded = jax.jit(
        shard_map(
            _body, mesh=mesh, in_specs=in_specs, out_specs=out_specs, check_rep=False
        ),
        donate_argnums=tuple(range(n_params, n_params + len(out_names))),
        keep_unused=True,
    )

    dev_in_cache = {}

    def run(in_maps, reuse_inputs=False):
        if reuse_inputs and "in" in dev_in_cache:
            concat_in = dev_in_cache["in"]
        else:
            concat_in = [
                np.concatenate([np.asarray(m[nm]) for m in in_maps], axis=0)
                for nm in in_names
            ]
            if reuse_inputs:
                from jax.sharding import NamedSharding

                sh = NamedSharding(mesh, PartitionSpec("core"))
                concat_in = [jax.device_put(a, sh) for a in concat_in]
                for a in concat_in:
                    a.block_until_ready()
                dev_in_cache["in"] = concat_in
        concat_zeros = [
            np.zeros((N_CORES * z.shape[0], *z.shape[1:]), z.dtype) for z in zero_outs
        ]
        out_arrs = sharded(*concat_in, *concat_zeros)
        out_arrs = [a.block_until_ready() for a in out_arrs]
        return [
            {
                nm: np.asarray(out_arrs[i]).reshape(N_CORES, *out_avals[i].shape)[c]
                for i, nm in enumerate(out_names)
            }
            for c in range(N_CORES)
        ]

    def time_loop(in_maps, n_calls):
        """Per-call wall times with inputs and donated zero-outputs pre-staged
        on device; outputs stay on device (only block_until_ready)."""
        import time as _t
        from jax.sharding import NamedSharding

        sh = NamedSharding(mesh, PartitionSpec("core"))
        concat_in = [
            jax.device_put(
                np.concatenate([np.asarray(m[nm]) for m in in_maps], axis=0), sh
            )
            for nm in in_names
        ]
        zeros_sets = [
            [
                jax.device_put(
                    np.zeros((N_CORES * z.shape[0], *z.shape[1:]), z.dtype), sh
                )
                for z in zero_outs
            ]
            for _ in range(n_calls)
        ]
        for a in concat_in:
            a.block_until_ready()
        for zs in zeros_sets:
            for a in zs:
                a.block_until_ready()
        # warm once (executable load)
        outs = sharded(*concat_in, *zeros_sets[0])
        [a.block_until_ready() for a in outs]
        times = []
        for i in range(1, n_calls):
            t0 = _t.perf_counter()
            outs = sharded(*concat_in, *zeros_sets[i])
            [a.block_until_ready() for a in outs]
            times.append(_t.perf_counter() - t0)
        return times

    def stage(in_maps, n_calls):
        """Pre-stage inputs + n_calls sets of donated zeros; return a closure
        that executes once per call (device exec + block)."""
        from jax.sharding import NamedSharding

        sh = NamedSharding(mesh, PartitionSpec("core"))
        concat_in = [
            jax.device_put(
                np.concatenate([np.asarray(m[nm]) for m in in_maps], axis=0), sh
            )
            for nm in in_names
        ]
        zeros_sets = [
            [
                jax.device_put(
                    np.zeros((N_CORES * z.shape[0], *z.shape[1:]), z.dtype), sh
                )
                for z in zero_outs
            ]
            for _ in range(n_calls)
        ]
        for a in concat_in:
            a.block_until_ready()
        for zs in zeros_sets:
            for a in zs:
                a.block_until_ready()
        state = {"i": 0}

        def call():
            i = state["i"]
            state["i"] += 1
            outs = sharded(*concat_in, *zeros_sets[i])
            # force full materialization — under the axon proxy,
            # block_until_ready alone does not wait for device execution
            return [np.asarray(a) for a in outs]

        return call

    def stage_async(in_maps, n_calls):
        """Like stage() but returns call(block=False) that does not wait."""
        from jax.sharding import NamedSharding

        sh = NamedSharding(mesh, PartitionSpec("core"))
        concat_in = [
            jax.device_put(
                np.concatenate([np.asarray(m[nm]) for m in in_maps], axis=0), sh
            )
            for nm in in_names
        ]
        zeros_sets = [
            [
                jax.device_put(
                    np.zeros((N_CORES * z.shape[0], *z.shape[1:]), z.dtype), sh
                )
                for z in zero_outs
            ]
            for _ in range(n_calls)
        ]
        for a in concat_in:
            a.block_until_ready()
        for zs in zeros_sets:
            for a in zs:
                a.block_until_ready()
        state = {"i": 0}

        def call(block=False):
            i = state["i"]
            state["i"] += 1
            outs = sharded(*concat_in, *zeros_sets[i])
            if block:
                outs = [np.asarray(a) for a in outs]
            return outs

        return call

    run.time_loop = time_loop
    run.stage = stage
    run.stage_async = stage_async
    _runner_cache[key] = run
    return run


N_CORES = 8
K = 12  # 6 basis terms x (hi, lo) coefficient rows
TILE_ROWS = 16
TILE_COLS = 32
F = TILE_ROWS * TILE_COLS  # 512 pixels per tile
QUADS = 4  # PE row-group rotation for arg matmul weight loads
FUSE = 2  # chunks per fused Exp activation
GROUP_TILES = 4  # tiles sharing one PSUM out bank via column groups

_prog_cache = {}


def _build_program(slot_nch, cutoff, repeat=1):
    """One SPMD program: 16 slots (tiles) per core, slot_nch[s] chunks each.

    cutoff is unused (kept for cache-key/test harness compatibility)."""
    n_slots = len(slot_nch)
    n_rounds = n_slots // GROUP_TILES
    tot = sum(slot_nch)
    X = tot * 128  # coef columns
    CB = X + F  # coef | basis columns
    nc = bacc.Bacc(
        "TRN2",
        target_bir_lowering=False,
        debug=False,
        enable_asserts=True,
        num_devices=N_CORES,
    )
    f32, f16 = mybir.dt.float32, mybir.dt.float16
    cb_ext = nc.dram_tensor("cb", [QUADS * K, CB], f16, kind="ExternalInput").ap()
    colors_ext = nc.dram_tensor("colors", [128, tot * 3], f16, kind="ExternalInput").ap()
    # 99 partition lines: 4 tile groups at partition offsets {0,32,64,96},
    # rows 3..31 of each group are don't-care (host reads rows 32*i..32*i+2)
    out_ext = nc.dram_tensor("out", [99, n_rounds * F], f16,
                             kind="ExternalOutput").ap()

    # flat chunk list: (slot, chunk-in-slot, global chunk idx)
    flat = []
    j = 0
    for s in range(n_slots):
        for c in range(slot_nch[s]):
            flat.append((s, c, j))
            j += 1
    groups = [flat[i : i + FUSE] for i in range(0, len(flat), FUSE)]

    with tile.TileContext(nc) as tc:
        with (
            tc.tile_pool(name="consts", bufs=1) as consts,
            tc.tile_pool(name="work", bufs=3) as work,
            tc.tile_pool(name="outsb", bufs=1) as outsb,
            tc.tile_pool(name="psum_arg", bufs=(3 if repeat == 1 else 2),
                         space="PSUM") as psum_arg,
            tc.tile_pool(name="psum_out", bufs=(2 if repeat == 1 else n_rounds),
                         space="PSUM") as psum_out,
        ):
            cb_sb = consts.tile([128, CB], f16, name="cb_sb")
            for q in range(QUADS):
                nc.sync.dma_start(
                    out=cb_sb[32 * q : 32 * q + K, :],
                    in_=cb_ext[q * K : (q + 1) * K, :],
                )
            colors_sb = consts.tile([128, tot * 3], f16, name="colors_sb")
            nc.sync.dma_start(out=colors_sb[:], in_=colors_ext[:])
            out_sb = outsb.tile([128, n_rounds * F], f16, name="out_sb")

            out_ps = {}  # round -> psum tile
            for rep in range(repeat):
                for g in groups:
                    glen = len(g)
                    arg_ps = psum_arg.tile([128, glen * F], f32, tag="arg")
                    for idx, (s, c, jj) in enumerate(g):
                        p0 = 32 * (jj % QUADS)
                        nc.tensor.matmul(
                            arg_ps[:, idx * F : (idx + 1) * F],
                            lhsT=cb_sb[p0 : p0 + K, jj * 128 : (jj + 1) * 128],
                            rhs=cb_sb[p0 : p0 + K, X : X + F],
                            start=True,
                            stop=True,
                            tile_position=(p0, 0),
                        )
                    alpha_sb = work.tile([128, glen * F], f16, tag="alpha")
                    nc.scalar.activation(
                        alpha_sb[:],
                        arg_ps[:],
                        mybir.ActivationFunctionType.Exp,
                    )
                    for idx, (s, c, jj) in enumerate(g):
                        r, i = divmod(s, GROUP_TILES)
                        if r not in out_ps:
                            out_ps[r] = psum_out.tile([99, F], f32, tag="out",
                                                      name=f"out_ps_{r}_{rep}")
                        p0 = 32 * i
                        nc.tensor.matmul(
                            out_ps[r][p0 : p0 + 3, :],
                            lhsT=colors_sb[:, jj * 3 : (jj + 1) * 3],
                            rhs=alpha_sb[:, idx * F : (idx + 1) * F],
                            start=(c == 0 and rep == 0),
                            stop=(c == slot_nch[s] - 1 and rep == repeat - 1),
                            tile_position=(0, p0),
                        )
                        # after the final chunk of a round's last slot, copy out
                        if (
                            rep == repeat - 1
                            and i == GROUP_TILES - 1
                            and c == slot_nch[s] - 1
                        ):
                            nc.vector.tensor_copy(
                                out_sb[0:99, r * F : (r + 1) * F], out_ps.pop(r)[:]
                            )
            nc.sync.dma_start(out=out_ext[:], in_=out_sb[0:99, :])
    nc.compile()
    return nc


def _get_program(slot_nch, cutoff, repeat=1):
    key = (tuple(slot_nch), float(cutoff), repeat)
    if key not in _prog_cache:
        _prog_cache[key] = _build_program(slot_nch, cutoff, repeat)
    return _prog_cache[key]


def _basis():
    """[6, F] f64 basis terms in tile-local coords (all fp16-exact)."""
    ys = np.arange(TILE_ROWS, dtype=np.float64) + 0.5 - TILE_ROWS / 2
    xs = np.arange(TILE_COLS, dtype=np.float64) + 0.5 - TILE_COLS / 2
    yl = np.repeat(ys, TILE_COLS)
    xl = np.tile(xs, TILE_ROWS)
    return np.stack([xl * xl, xl * yl, yl * yl, xl, yl, np.ones_like(xl)], axis=0)


def kernel(
    opacity,
    means,
    stds,
    rhos,
    colors,
    image_height,
    image_width,
    scale_factor,
    raster_ratio,
    _repeat=1,
    _time_exec=False,
    _bench_calls=0,
):
    H = int(image_height)
    W = int(image_width)
    sf = float(scale_factor)
    rr = float(raster_ratio)
    opacity = np.asarray(opacity, np.float64)
    means = np.asarray(means, np.float64)
    stds = np.asarray(stds, np.float64) * sf
    rhos = np.asarray(rhos, np.float64)
    colors = np.asarray(colors, np.float32)
    N = opacity.shape[0]

    n_ty = H // TILE_ROWS
    n_tx = W // TILE_COLS
    n_tiles = n_ty * n_tx
    assert n_tiles % N_CORES == 0
    n_slots = n_tiles // N_CORES
    assert n_slots % GROUP_TILES == 0

    # --- per-gaussian inverse covariance (f64)
    sx, sy = stds[:, 0], stds[:, 1]
    om = 1.0 - rhos * rhos
    ia = 1.0 / (sx * sx * om)
    ib = -rhos / (sx * sy * om)
    ic = 1.0 / (sy * sy * om)
    mx, my = means[:, 0], means[:, 1]
    lnop = np.log(np.maximum(opacity, 1e-30))

    # --- exact ellipse-vs-rect cull: min Mahalanobis^2 over pixel centers
    cut2 = rr * rr + 1e-6

    def min_m2(x0, x1, y0, y1):
        dx0, dx1 = x0 - mx, x1 - mx
        dy0, dy1 = y0 - my, y1 - my
        inside = (dx0 <= 0) & (dx1 >= 0) & (dy0 <= 0) & (dy1 >= 0)
        best = np.full(N, np.inf)
        for cdx in (dx0, dx1):
            dy = np.clip(-ib * cdx / ic, dy0, dy1)
            best = np.minimum(best, ia * cdx * cdx + 2 * ib * cdx * dy + ic * dy * dy)
        for cdy in (dy0, dy1):
            dx = np.clip(-ib * cdy / ia, dx0, dx1)
            best = np.minimum(best, ia * dx * dx + 2 * ib * cdy * dx + ic * cdy * cdy)
        return np.where(inside, 0.0, best)

    tile_ids = []
    tile_pos = []
    for tyi in range(n_ty):
        ty = tyi * TILE_ROWS
        for txi in range(n_tx):
            tx = txi * TILE_COLS
            m2 = min_m2(tx + 0.5, tx + TILE_COLS - 0.5, ty + 0.5, ty + TILE_ROWS - 0.5)
            tile_ids.append(np.nonzero(m2 <= cut2)[0])
            tile_pos.append((ty, tx))

    # snake-deal tiles to cores by descending chunk need so the SPMD slot
    # capacities (max over cores per slot) hug each core's real need
    nchs = [max(1, (len(ids) + 127) // 128) for ids in tile_ids]
    t_order = sorted(range(n_tiles), key=lambda t: -nchs[t])
    assign = [[] for _ in range(N_CORES)]
    for i, t in enumerate(t_order):
        rnd, pos = divmod(i, N_CORES)
        core = pos if rnd % 2 == 0 else N_CORES - 1 - pos
        assign[core].append(t)
    slot_nch = tuple(
        max(nchs[assign[core][k]] for core in range(N_CORES)) for k in range(n_slots)
    )
    tot = sum(slot_nch)
    X = tot * 128

    nc = _get_program(slot_nch, 0.0, _repeat)

    basis6 = _basis()  # [6, F] f64, fp16-exact values

    in_maps = []
    for core in range(N_CORES):
        cb_arr = np.zeros((QUADS * K, X + F), np.float16)
        colors_arr = np.zeros((128, tot * 3), np.float16)
        base = 0
        for k in range(n_slots):
            t = assign[core][k]
            ty, tx = tile_pos[t]
            ids = tile_ids[t]
            gn = len(ids)
            assert gn <= slot_nch[k] * 128
            if gn:
                cxo = tx + TILE_COLS / 2
                cyo = ty + TILE_ROWS / 2
                mxl = mx[ids] - cxo
                myl = my[ids] - cyo
                A, B, C = ia[ids], ib[ids], ic[ids]
                cf = np.stack(
                    [
                        -0.5 * A,
                        -B,
                        -0.5 * C,
                        A * mxl + B * myl,
                        B * mxl + C * myl,
                        -0.5 * (A * mxl * mxl + 2 * B * mxl * myl + C * myl * myl)
                        + lnop[ids],
                    ],
                    axis=0,
                )  # [6, gn] f64
                hi = cf.astype(np.float16)
                lo = (cf - hi.astype(np.float64)).astype(np.float16)
                col = colors[ids].astype(np.float16)
                for c in range((gn + 127) // 128):
                    lo_i, hi_i = c * 128, min((c + 1) * 128, gn)
                    jj = base + c
                    for q in range(QUADS):
                        cb_arr[q * K : q * K + 6, jj * 128 : jj * 128 + hi_i - lo_i] = (
                            hi[:, lo_i:hi_i]
                        )
                        cb_arr[q * K + 6 : q * K + 12,
                               jj * 128 : jj * 128 + hi_i - lo_i] = lo[:, lo_i:hi_i]
                    colors_arr[: hi_i - lo_i, jj * 3 : jj * 3 + 3] = col[lo_i:hi_i]
            base += slot_nch[k]
        for q in range(QUADS):
            cb_arr[q * K : q * K + 6, X : X + F] = basis6
            cb_arr[q * K + 6 : q * K + 12, X : X + F] = basis6
        in_maps.append({"cb": cb_arr, "colors": colors_arr})

    import time as _time

    global _last_in_maps
    _last_in_maps = in_maps
    run = _get_runner(nc)
    if _bench_calls:
        return run.time_loop(in_maps, _bench_calls)
    t0 = _time.time()
    results = run(in_maps, reuse_inputs=_time_exec)
    exec_wall = _time.time() - t0

    out = np.zeros((H, W, 3), np.float32)
    for core in range(N_CORES):
        o = results[core]["out"]  # [99, n_rounds*F] f16; rows 32i..32i+2 real
        for k in range(n_slots):
            ty, tx = tile_pos[assign[core][k]]
            r, i = divmod(k, GROUP_TILES)
            blk = o[32 * i : 32 * i + 3, r * F : (r + 1) * F].astype(
                np.float32
            ).reshape(3, TILE_ROWS, TILE_COLS)
            out[ty : ty + TILE_ROWS, tx : tx + TILE_COLS, :] = blk.transpose(1, 2, 0)
    if _repeat > 1:
        out /= np.float32(_repeat)
    if _time_exec:
        return out, exec_wall
    return out


# revision 19
# speedup vs baseline: 2.9832x; 1.0436x over previous
"""2D Gaussian splat rasterizer on 8 Trainium2 NeuronCores.

Strategy: shard the image into 128 tiles of 16x32 px (F=512), dealt to 8
cores (16 slots each). Gaussians are culled host-side per tile by the
EXACT min-Mahalanobis-over-rect test (<= raster_ratio^2) and packed into
chunks of <=128 (partition dim). Per chunk, on device:

    arg   = coefT.T @ basis      TensorE, K=12 fp16, 1 cy/row: the 6-term
                                 pixel-basis [x2, xy, y2, x, y, 1] quadratic
                                 (tile-local coords) with each coefficient
                                 split hi/lo across two fp16 rows (full-f32
                                 effective precision; basis values are all
                                 exactly representable in fp16), constant
                                 row folds in ln(opacity)
    alpha = Exp(arg)             ScalarE from PSUM, fp16 out, fused over
                                 FUSE consecutive chunks (no bias, no mask:
                                 the cutoff is applied by the exact cull,
                                 in-tile tails are within tolerance)
    out  += colors.T @ alpha     TensorE, K=128 fp16, PSUM accumulate;
                                 4 tiles share one PSUM bank via
                                 tile_position column groups {0,32,64,96}

Per round of 4 tiles one DVE copy moves the PSUM bank to fp16 SBUF; one
final DMA (partition-strided AP) writes all 16 tiles out. 3 DMAs total.
The full [H, W, 3] image is reassembled host-side (no collectives).
"""

import numpy as np
import concourse.bacc as bacc
import concourse.tile as tile
from concourse import mybir
from concourse.bass_utils import run_bass_kernel_spmd

_runner_cache = {}


def _get_runner(nc):
    """Persistent jitted SPMD executor for a compiled Bass program (modeled on
    bass2jax.run_bass_via_pjrt's multi-core path, but cached so repeat calls
    reuse the same XLA executable — no retrace, no NEFF reload)."""
    key = id(nc)
    if key in _runner_cache:
        return _runner_cache[key]
    import jax
    import jax.numpy as jnp
    from jax.sharding import Mesh, PartitionSpec
    from jax.experimental.shard_map import shard_map
    from concourse import bass2jax, mybir as mb

    bass2jax.install_neuronx_cc_hook()

    in_names, out_names, out_avals, zero_outs = [], [], [], []
    partition_name = nc.partition_id_tensor.name if nc.partition_id_tensor else None
    for alloc in nc.m.functions[0].allocations:
        if not isinstance(alloc, mb.MemoryLocationSet):
            continue
        name = alloc.memorylocations[0].name
        if alloc.kind == "ExternalInput":
            if name != partition_name:
                in_names.append(name)
        elif alloc.kind == "ExternalOutput":
            shape = tuple(alloc.tensor_shape)
            dtype = mb.dt.np(alloc.dtype)
            out_names.append(name)
            out_avals.append(jax.core.ShapedArray(shape, dtype))
            zero_outs.append(np.zeros(shape, dtype))
    n_params = len(in_names)
    all_in = in_names + out_names + ([partition_name] if partition_name else [])

    def _body(*args):
        operands = list(args)
        if partition_name is not None:
            operands.append(bass2jax.partition_id_tensor())
        outs = bass2jax._bass_exec_p.bind(
            *operands,
            out_avals=tuple(out_avals),
            in_names=tuple(all_in),
            out_names=tuple(out_names),
            lowering_input_output_aliases=(),
            sim_require_finite=True,
            sim_require_nnan=True,
            nc=nc,
        )
        return tuple(outs)

    devices = jax.devices()[:N_CORES]
    mesh = Mesh(np.asarray(devices), ("core",))
    in_specs = (PartitionSpec("core"),) * (n_params + len(out_names))
    out_specs = (PartitionSpec("core"),) * len(out_names)
    sharded = jax.jit(
        shard_map(
            _body, mesh=mesh, in_specs=in_specs, out_specs=out_specs, check_rep=False
        ),
        donate_argnums=tuple(range(n_params, n_params + len(out_names))),
        keep_unused=True,
    )

    dev_in_cache = {}

    def run(in_maps, reuse_inputs=False):
        if reuse_inputs and "in" in dev_in_cache:
            concat_in = dev_in_cache["in"]
        else:
            concat_in = [
                np.concatenate([np.asarray(m[nm]) for m in in_maps], axis=0)
                for nm in in_names
            ]
            if reuse_inputs:
                from jax.sharding import NamedSharding

                sh = NamedSharding(mesh, PartitionSpec("core"))
                concat_in = [jax.device_put(a, sh) for a in concat_in]
                for a in concat_in:
                    a.block_until_ready()
                dev_in_cache["in"] = concat_in
        concat_zeros = [
            np.zeros((N_CORES * z.shape[0], *z.shape[1:]), z.dtype) for z in zero_outs
        ]
        out_arrs = sharded(*concat_in, *concat_zeros)
        out_arrs = [a.block_until_ready() for a in out_arrs]
        return [
            {
                nm: np.asarray(out_arrs[i]).reshape(N_CORES, *out_avals[i].shape)[c]
                for i, nm in enumerate(out_names)
            }
            for c in range(N_CORES)
        ]

    def time_loop(in_maps, n_calls):
        """Per-call wall times with inputs and donated zero-outputs pre-staged
        on device; outputs stay on device (only block_until_ready)."""
        import time as _t
        from jax.sharding import NamedSharding

        sh = NamedSharding(mesh, PartitionSpec("core"))
        concat_in = [
            jax.device_put(
                np.concatenate([np.asarray(m[nm]) for m in in_maps], axis=0), sh
            )
            for nm in in_names
        ]
        zeros_sets = [
            [
                jax.device_put(
                    np.zeros((N_CORES * z.shape[0], *z.shape[1:]), z.dtype), sh
                )
                for z in zero_outs
            ]
            for _ in range(n_calls)
        ]
        for a in concat_in:
            a.block_until_ready()
        for zs in zeros_sets:
            for a in zs:
                a.block_until_ready()
        # warm once (executable load)
        outs = sharded(*concat_in, *zeros_sets[0])
        [a.block_until_ready() for a in outs]
        times = []
        for i in range(1, n_calls):
            t0 = _t.perf_counter()
            outs = sharded(*concat_in, *zeros_sets[i])
            [a.block_until_ready() for a in outs]
            times.append(_t.perf_counter() - t0)
        return times

    def stage(in_maps, n_calls):
        """Pre-stage inputs + n_calls sets of donated zeros; return a closure
        that executes once per call (device exec + block)."""
        from jax.sharding import NamedSharding

        sh = NamedSharding(mesh, PartitionSpec("core"))
        concat_in = [
            jax.device_put(
                np.concatenate([np.asarray(m[nm]) for m in in_maps], axis=0), sh
            )
            for nm in in_names
        ]
        zeros_sets = [
            [
                jax.device_put(
                    np.zeros((N_CORES * z.shape[0], *z.shape[1:]), z.dtype), sh
                )
                for z in zero_outs
            ]
            for _ in range(n_calls)
        ]
        for a in concat_in:
            a.block_until_ready()
        for zs in zeros_sets:
            for a in zs:
                a.block_until_ready()
        state = {"i": 0}

        def call():
            i = state["i"]
            state["i"] += 1
            outs = sharded(*concat_in, *zeros_sets[i])
            # force full materialization — under the axon proxy,
            # block_until_ready alone does not wait for device execution
            return [np.asarray(a) for a in outs]

        return call

    def stage_async(in_maps, n_calls):
        """Like stage() but returns call(block=False) that does not wait."""
        from jax.sharding import NamedSharding

        sh = NamedSharding(mesh, PartitionSpec("core"))
        concat_in = [
            jax.device_put(
                np.concatenate([np.asarray(m[nm]) for m in in_maps], axis=0), sh
            )
            for nm in in_names
        ]
        zeros_sets = [
            [
                jax.device_put(
                    np.zeros((N_CORES * z.shape[0], *z.shape[1:]), z.dtype), sh
                )
                for z in zero_outs
            ]
            for _ in range(n_calls)
        ]
        for a in concat_in:
            a.block_until_ready()
        for zs in zeros_sets:
            for a in zs:
                a.block_until_ready()
        state = {"i": 0}

        def call(block=False):
            i = state["i"]
            state["i"] += 1
            outs = sharded(*concat_in, *zeros_sets[i])
            if block:
                outs = [np.asarray(a) for a in outs]
            return outs

        return call

    run.time_loop = time_loop
    run.stage = stage
    run.stage_async = stage_async
    _runner_cache[key] = run
    return run


N_CORES = 8
K = 12  # 6 basis terms x (hi, lo) coefficient rows
TILE_ROWS = 16
TILE_COLS = 32
F = TILE_ROWS * TILE_COLS  # 512 pixels per tile
QUADS = 4  # PE row-group rotation for arg matmul weight loads
FUSE = 3  # chunks per fused Exp activation
GROUP_TILES = 4  # tiles sharing one PSUM out bank via column groups

_prog_cache = {}


def _build_program(slot_nch, cutoff, repeat=1):
    """One SPMD program: 16 slots (tiles) per core, slot_nch[s] chunks each.

    cutoff is unused (kept for cache-key/test harness compatibility)."""
    n_slots = len(slot_nch)
    n_rounds = n_slots // GROUP_TILES
    tot = sum(slot_nch)
    X = tot * 128  # coef columns
    CB = X + F  # coef | basis columns
    nc = bacc.Bacc(
        "TRN2",
        target_bir_lowering=False,
        debug=False,
        enable_asserts=True,
        num_devices=N_CORES,
    )
    f32, f16 = mybir.dt.float32, mybir.dt.float16
    cb_ext = nc.dram_tensor("cb", [QUADS, K, CB], f16, kind="ExternalInput").ap()
    colors_ext = nc.dram_tensor("colors", [128, tot * 3], f16, kind="ExternalInput").ap()
    # 99 partition lines: 4 tile groups at partition offsets {0,32,64,96},
    # rows 3..31 of each group are don't-care (host reads rows 32*i..32*i+2)
    out_ext = nc.dram_tensor("out", [99, n_rounds * F], f16,
                             kind="ExternalOutput").ap()

    # per-round flat chunk lists: (slot, chunk-in-slot, global chunk idx)
    j = 0
    round_flat = []
    for r in range(n_rounds):
        fl = []
        for s in range(r * GROUP_TILES, (r + 1) * GROUP_TILES):
            for c in range(slot_nch[s]):
                fl.append((s, c, j))
                j += 1
        round_flat.append(fl)

    with tile.TileContext(nc) as tc:
        with (
            tc.tile_pool(name="consts", bufs=1) as consts,
            tc.tile_pool(name="work", bufs=3) as work,
            tc.tile_pool(name="outsb", bufs=1) as outsb,
            tc.tile_pool(name="psum_arg", bufs=2, space="PSUM") as psum_arg,
            tc.tile_pool(name="psum_out", bufs=2, space="PSUM") as psum_out,
        ):
            cb_sb = consts.tile([128, CB], f16, name="cb_sb")
            for q in range(QUADS):
                nc.sync.dma_start(
                    out=cb_sb[32 * q : 32 * q + K, :], in_=cb_ext[q]
                )
            colors_sb = consts.tile([128, tot * 3], f16, name="colors_sb")
            nc.sync.dma_start(out=colors_sb[:], in_=colors_ext[:])
            out_sb = outsb.tile([128, n_rounds * F], f16, name="out_sb")

            for r in range(n_rounds):
                out_ps = psum_out.tile([99, F], f32, tag="out", name=f"out_ps_{r}")
                for rep in range(repeat):
                    fl = round_flat[r]
                    for g0 in range(0, len(fl), FUSE):
                        g = fl[g0 : g0 + FUSE]
                        glen = len(g)
                        arg_ps = psum_arg.tile([128, glen * F], f32, tag="arg")
                        for idx, (s, c, jj) in enumerate(g):
                            p0 = 32 * (jj % QUADS)
                            nc.tensor.matmul(
                                arg_ps[:, idx * F : (idx + 1) * F],
                                lhsT=cb_sb[p0 : p0 + K, jj * 128 : (jj + 1) * 128],
                                rhs=cb_sb[p0 : p0 + K, X : X + F],
                                start=True,
                                stop=True,
                                tile_position=(p0, 0),
                            )
                        alpha_sb = work.tile([128, glen * F], f16, tag="alpha")
                        nc.scalar.activation(
                            alpha_sb[:],
                            arg_ps[:],
                            mybir.ActivationFunctionType.Exp,
                        )
                        for idx, (s, c, jj) in enumerate(g):
                            i = s % GROUP_TILES
                            p0 = 32 * i
                            nc.tensor.matmul(
                                out_ps[p0 : p0 + 3, :],
                                lhsT=colors_sb[:, jj * 3 : (jj + 1) * 3],
                                rhs=alpha_sb[:, idx * F : (idx + 1) * F],
                                start=(c == 0 and rep == 0),
                                stop=(c == slot_nch[s] - 1 and rep == repeat - 1),
                                tile_position=(0, p0),
                            )
                nc.vector.tensor_copy(out_sb[0:99, r * F : (r + 1) * F], out_ps[:])
            nc.sync.dma_start(out=out_ext[:], in_=out_sb[0:99, :])
    nc.compile()
    return nc


def _get_program(slot_nch, cutoff, repeat=1):
    key = (tuple(slot_nch), float(cutoff), repeat)
    if key not in _prog_cache:
        _prog_cache[key] = _build_program(slot_nch, cutoff, repeat)
    return _prog_cache[key]


def _basis():
    """[6, F] f64 basis terms in tile-local coords (all fp16-exact)."""
    ys = np.arange(TILE_ROWS, dtype=np.float64) + 0.5 - TILE_ROWS / 2
    xs = np.arange(TILE_COLS, dtype=np.float64) + 0.5 - TILE_COLS / 2
    yl = np.repeat(ys, TILE_COLS)
    xl = np.tile(xs, TILE_ROWS)
    return np.stack([xl * xl, xl * yl, yl * yl, xl, yl, np.ones_like(xl)], axis=0)


def kernel(
    opacity,
    means,
    stds,
    rhos,
    colors,
    image_height,
    image_width,
    scale_factor,
    raster_ratio,
    _repeat=1,
    _time_exec=False,
    _bench_calls=0,
):
    H = int(image_height)
    W = int(image_width)
    sf = float(scale_factor)
    rr = float(raster_ratio)
    opacity = np.asarray(opacity, np.float64)
    means = np.asarray(means, np.float64)
    stds = np.asarray(stds, np.float64) * sf
    rhos = np.asarray(rhos, np.float64)
    colors = np.asarray(colors, np.float32)
    N = opacity.shape[0]

    n_ty = H // TILE_ROWS
    n_tx = W // TILE_COLS
    n_tiles = n_ty * n_tx
    assert n_tiles % N_CORES == 0
    n_slots = n_tiles // N_CORES
    assert n_slots % GROUP_TILES == 0

    # --- per-gaussian inverse covariance (f64)
    sx, sy = stds[:, 0], stds[:, 1]
    om = 1.0 - rhos * rhos
    ia = 1.0 / (sx * sx * om)
    ib = -rhos / (sx * sy * om)
    ic = 1.0 / (sy * sy * om)
    mx, my = means[:, 0], means[:, 1]
    lnop = np.log(np.maximum(opacity, 1e-30))

    # --- exact ellipse-vs-rect cull: min Mahalanobis^2 over pixel centers
    cut2 = rr * rr + 1e-6

    def min_m2(x0, x1, y0, y1):
        dx0, dx1 = x0 - mx, x1 - mx
        dy0, dy1 = y0 - my, y1 - my
        inside = (dx0 <= 0) & (dx1 >= 0) & (dy0 <= 0) & (dy1 >= 0)
        best = np.full(N, np.inf)
        for cdx in (dx0, dx1):
            dy = np.clip(-ib * cdx / ic, dy0, dy1)
            best = np.minimum(best, ia * cdx * cdx + 2 * ib * cdx * dy + ic * dy * dy)
        for cdy in (dy0, dy1):
            dx = np.clip(-ib * cdy / ia, dx0, dx1)
            best = np.minimum(best, ia * dx * dx + 2 * ib * cdy * dx + ic * cdy * cdy)
        return np.where(inside, 0.0, best)

    tile_ids = []
    tile_pos = []
    for tyi in range(n_ty):
        ty = tyi * TILE_ROWS
        for txi in range(n_tx):
            tx = txi * TILE_COLS
            m2 = min_m2(tx + 0.5, tx + TILE_COLS - 0.5, ty + 0.5, ty + TILE_ROWS - 0.5)
            tile_ids.append(np.nonzero(m2 <= cut2)[0])
            tile_pos.append((ty, tx))

    # snake-deal tiles to cores by descending chunk need so the SPMD slot
    # capacities (max over cores per slot) hug each core's real need
    nchs = [max(1, (len(ids) + 127) // 128) for ids in tile_ids]
    t_order = sorted(range(n_tiles), key=lambda t: -nchs[t])
    assign = [[] for _ in range(N_CORES)]
    for i, t in enumerate(t_order):
        rnd, pos = divmod(i, N_CORES)
        core = pos if rnd % 2 == 0 else N_CORES - 1 - pos
        assign[core].append(t)
    # permute slots so round chunk-counts hug multiples of FUSE (heavy slots
    # paired with light slots within a round)
    perm = []
    lo, hi = 0, n_slots - 1
    while lo < hi:
        perm.extend([lo, lo + 1, hi - 1, hi])
        lo += 2
        hi -= 2
    assign = [[a[p] for p in perm] for a in assign]
    slot_nch = tuple(
        max(nchs[assign[core][k]] for core in range(N_CORES)) for k in range(n_slots)
    )
    tot = sum(slot_nch)
    X = tot * 128

    nc = _get_program(slot_nch, 0.0, _repeat)

    basis6 = _basis()  # [6, F] f64, fp16-exact values

    in_maps = []
    for core in range(N_CORES):
        cb_arr = np.zeros((QUADS, K, X + F), np.float16)
        colors_arr = np.zeros((128, tot * 3), np.float16)
        base = 0
        for k in range(n_slots):
            t = assign[core][k]
            ty, tx = tile_pos[t]
            ids = tile_ids[t]
            gn = len(ids)
            assert gn <= slot_nch[k] * 128
            if gn:
                cxo = tx + TILE_COLS / 2
                cyo = ty + TILE_ROWS / 2
                mxl = mx[ids] - cxo
                myl = my[ids] - cyo
                A, B, C = ia[ids], ib[ids], ic[ids]
                cf = np.stack(
                    [
                        -0.5 * A,
                        -B,
                        -0.5 * C,
                        A * mxl + B * myl,
                        B * mxl + C * myl,
                        -0.5 * (A * mxl * mxl + 2 * B * mxl * myl + C * myl * myl)
                        + lnop[ids],
                    ],
                    axis=0,
                )  # [6, gn] f64
                hi = cf.astype(np.float16)
                lo = (cf - hi.astype(np.float64)).astype(np.float16)
                col = colors[ids].astype(np.float16)
                for c in range((gn + 127) // 128):
                    lo_i, hi_i = c * 128, min((c + 1) * 128, gn)
                    jj = base + c
                    cb_arr[:, 0:6, jj * 128 : jj * 128 + hi_i - lo_i] = hi[:, lo_i:hi_i]
                    cb_arr[:, 6:12, jj * 128 : jj * 128 + hi_i - lo_i] = lo[:, lo_i:hi_i]
                    colors_arr[: hi_i - lo_i, jj * 3 : jj * 3 + 3] = col[lo_i:hi_i]
            base += slot_nch[k]
        cb_arr[:, 0:6, X : X + F] = basis6
        cb_arr[:, 6:12, X : X + F] = basis6
        in_maps.append({"cb": cb_arr, "colors": colors_arr})

    import time as _time

    global _last_in_maps
    _last_in_maps = in_maps
    run = _get_runner(nc)
    if _bench_calls:
        return run.time_loop(in_maps, _bench_calls)
    t0 = _time.time()
    results = run(in_maps, reuse_inputs=_time_exec)
    exec_wall = _time.time() - t0

    out = np.zeros((H, W, 3), np.float32)
    for core in range(N_CORES):
        o = results[core]["out"]  # [99, n_rounds*F] f16; rows 32i..32i+2 real
        for k in range(n_slots):
            ty, tx = tile_pos[assign[core][k]]
            r, i = divmod(k, GROUP_TILES)
            blk = o[32 * i : 32 * i + 3, r * F : (r + 1) * F].astype(
                np.float32
            ).reshape(3, TILE_ROWS, TILE_COLS)
            out[ty : ty + TILE_ROWS, tx : tx + TILE_COLS, :] = blk.transpose(1, 2, 0)
    if _repeat > 1:
        out /= np.float32(_repeat)
    if _time_exec:
        return out, exec_wall
    return out


# revision 21
# speedup vs baseline: 3.5099x; 1.1766x over previous
"""2D Gaussian splat rasterizer on 8 Trainium2 NeuronCores.

Strategy: shard the image into 128 tiles of 16x32 px (F=512), dealt to 8
cores (16 slots each). Gaussians are culled host-side per tile by the
EXACT min-Mahalanobis-over-rect test (<= raster_ratio^2) and packed into
chunks of <=128 (partition dim). Per chunk, on device:

    arg   = coefT.T @ basis      TensorE, K=12 fp16, 1 cy/row: the 6-term
                                 pixel-basis [x2, xy, y2, x, y, 1] quadratic
                                 (tile-local coords) with each coefficient
                                 split hi/lo across two fp16 rows (full-f32
                                 effective precision; basis values are all
                                 exactly representable in fp16), constant
                                 row folds in ln(opacity)
    alpha = Exp(arg)             ScalarE from PSUM, fp16 out, fused over
                                 FUSE consecutive chunks (no bias, no mask:
                                 the cutoff is applied by the exact cull,
                                 in-tile tails are within tolerance)
    out  += colors.T @ alpha     TensorE, K=128 fp16, PSUM accumulate;
                                 4 tiles share one PSUM bank via
                                 tile_position column groups {0,32,64,96}

Per round of 4 tiles one DVE copy moves the PSUM bank to fp16 SBUF; one
final DMA (partition-strided AP) writes all 16 tiles out. 3 DMAs total.
The full [H, W, 3] image is reassembled host-side (no collectives).
"""

import numpy as np
import concourse.bacc as bacc
import concourse.tile as tile
from concourse import mybir
from concourse.bass_utils import run_bass_kernel_spmd

_runner_cache = {}


def _get_runner(nc):
    """Persistent jitted SPMD executor for a compiled Bass program (modeled on
    bass2jax.run_bass_via_pjrt's multi-core path, but cached so repeat calls
    reuse the same XLA executable — no retrace, no NEFF reload)."""
    key = id(nc)
    if key in _runner_cache:
        return _runner_cache[key]
    import jax
    import jax.numpy as jnp
    from jax.sharding import Mesh, PartitionSpec
    from jax.experimental.shard_map import shard_map
    from concourse import bass2jax, mybir as mb

    bass2jax.install_neuronx_cc_hook()

    in_names, out_names, out_avals, zero_outs = [], [], [], []
    partition_name = nc.partition_id_tensor.name if nc.partition_id_tensor else None
    for alloc in nc.m.functions[0].allocations:
        if not isinstance(alloc, mb.MemoryLocationSet):
            continue
        name = alloc.memorylocations[0].name
        if alloc.kind == "ExternalInput":
            if name != partition_name:
                in_names.append(name)
        elif alloc.kind == "ExternalOutput":
            shape = tuple(alloc.tensor_shape)
            dtype = mb.dt.np(alloc.dtype)
            out_names.append(name)
            out_avals.append(jax.core.ShapedArray(shape, dtype))
            zero_outs.append(np.zeros(shape, dtype))
    n_params = len(in_names)
    all_in = in_names + out_names + ([partition_name] if partition_name else [])

    def _body(*args):
        operands = list(args)
        if partition_name is not None:
            operands.append(bass2jax.partition_id_tensor())
        outs = bass2jax._bass_exec_p.bind(
            *operands,
            out_avals=tuple(out_avals),
            in_names=tuple(all_in),
            out_names=tuple(out_names),
            lowering_input_output_aliases=(),
            sim_require_finite=True,
            sim_require_nnan=True,
            nc=nc,
        )
        return tuple(outs)

    devices = jax.devices()[:N_CORES]
    mesh = Mesh(np.asarray(devices), ("core",))
    in_specs = (PartitionSpec("core"),) * (n_params + len(out_names))
    out_specs = (PartitionSpec("core"),) * len(out_names)
    sharded = jax.jit(
        shard_map(
            _body, mesh=mesh, in_specs=in_specs, out_specs=out_specs, check_rep=False
        ),
        donate_argnums=tuple(range(n_params, n_params + len(out_names))),
        keep_unused=True,
    )

    dev_in_cache = {}

    def run(in_maps, reuse_inputs=False):
        if reuse_inputs and "in" in dev_in_cache:
            concat_in = dev_in_cache["in"]
        else:
            concat_in = [
                np.concatenate([np.asarray(m[nm]) for m in in_maps], axis=0)
                for nm in in_names
            ]
            if reuse_inputs:
                from jax.sharding import NamedSharding

                sh = NamedSharding(mesh, PartitionSpec("core"))
                concat_in = [jax.device_put(a, sh) for a in concat_in]
                for a in concat_in:
                    a.block_until_ready()
                dev_in_cache["in"] = concat_in
        concat_zeros = [
            np.zeros((N_CORES * z.shape[0], *z.shape[1:]), z.dtype) for z in zero_outs
        ]
        out_arrs = sharded(*concat_in, *concat_zeros)
        out_arrs = [a.block_until_ready() for a in out_arrs]
        return [
            {
                nm: np.asarray(out_arrs[i]).reshape(N_CORES, *out_avals[i].shape)[c]
                for i, nm in enumerate(out_names)
            }
            for c in range(N_CORES)
        ]

    def time_loop(in_maps, n_calls):
        """Per-call wall times with inputs and donated zero-outputs pre-staged
        on device; outputs stay on device (only block_until_ready)."""
        import time as _t
        from jax.sharding import NamedSharding

        sh = NamedSharding(mesh, PartitionSpec("core"))
        concat_in = [
            jax.device_put(
                np.concatenate([np.asarray(m[nm]) for m in in_maps], axis=0), sh
            )
            for nm in in_names
        ]
        zeros_sets = [
            [
                jax.device_put(
                    np.zeros((N_CORES * z.shape[0], *z.shape[1:]), z.dtype), sh
                )
                for z in zero_outs
            ]
            for _ in range(n_calls)
        ]
        for a in concat_in:
            a.block_until_ready()
        for zs in zeros_sets:
            for a in zs:
                a.block_until_ready()
        # warm once (executable load)
        outs = sharded(*concat_in, *zeros_sets[0])
        [a.block_until_ready() for a in outs]
        times = []
        for i in range(1, n_calls):
            t0 = _t.perf_counter()
            outs = sharded(*concat_in, *zeros_sets[i])
            [a.block_until_ready() for a in outs]
            times.append(_t.perf_counter() - t0)
        return times

    def stage(in_maps, n_calls):
        """Pre-stage inputs + n_calls sets of donated zeros; return a closure
        that executes once per call (device exec + block)."""
        from jax.sharding import NamedSharding

        sh = NamedSharding(mesh, PartitionSpec("core"))
        concat_in = [
            jax.device_put(
                np.concatenate([np.asarray(m[nm]) for m in in_maps], axis=0), sh
            )
            for nm in in_names
        ]
        zeros_sets = [
            [
                jax.device_put(
                    np.zeros((N_CORES * z.shape[0], *z.shape[1:]), z.dtype), sh
                )
                for z in zero_outs
            ]
            for _ in range(n_calls)
        ]
        for a in concat_in:
            a.block_until_ready()
        for zs in zeros_sets:
            for a in zs:
                a.block_until_ready()
        state = {"i": 0}

        def call():
            i = state["i"]
            state["i"] += 1
            outs = sharded(*concat_in, *zeros_sets[i])
            # force full materialization — under the axon proxy,
            # block_until_ready alone does not wait for device execution
            return [np.asarray(a) for a in outs]

        return call

    def stage_async(in_maps, n_calls):
        """Like stage() but returns call(block=False) that does not wait."""
        from jax.sharding import NamedSharding

        sh = NamedSharding(mesh, PartitionSpec("core"))
        concat_in = [
            jax.device_put(
                np.concatenate([np.asarray(m[nm]) for m in in_maps], axis=0), sh
            )
            for nm in in_names
        ]
        zeros_sets = [
            [
                jax.device_put(
                    np.zeros((N_CORES * z.shape[0], *z.shape[1:]), z.dtype), sh
                )
                for z in zero_outs
            ]
            for _ in range(n_calls)
        ]
        for a in concat_in:
            a.block_until_ready()
        for zs in zeros_sets:
            for a in zs:
                a.block_until_ready()
        state = {"i": 0}

        def call(block=False):
            i = state["i"]
            state["i"] += 1
            outs = sharded(*concat_in, *zeros_sets[i])
            if block:
                outs = [np.asarray(a) for a in outs]
            return outs

        return call

    run.time_loop = time_loop
    run.stage = stage
    run.stage_async = stage_async
    _runner_cache[key] = run
    return run


N_CORES = 8
K = 12  # 6 basis terms x (hi, lo) coefficient rows
TILE_ROWS = 16
TILE_COLS = 32
F = TILE_ROWS * TILE_COLS  # 512 pixels per tile
QUADS = 4  # PE row-group rotation for arg matmul weight loads
FUSE = 3  # chunks per fused Exp activation
GROUP_TILES = 4  # tiles sharing one PSUM out bank via column groups

_prog_cache = {}


def _build_program(slot_nch, cutoff, repeat=1):
    """One SPMD program: 16 slots (tiles) per core, slot_nch[s] chunks each.

    cutoff is unused (kept for cache-key/test harness compatibility)."""
    n_slots = len(slot_nch)
    n_rounds = n_slots // GROUP_TILES
    tot = sum(slot_nch)
    X = tot * 128  # coef columns
    CB = X + F  # coef | basis columns
    nc = bacc.Bacc(
        "TRN2",
        target_bir_lowering=False,
        debug=False,
        enable_asserts=True,
        num_devices=N_CORES,
    )
    f32, f16 = mybir.dt.float32, mybir.dt.float16
    cb_ext = nc.dram_tensor("cb", [QUADS, K, CB], f16, kind="ExternalInput").ap()
    colors_ext = nc.dram_tensor("colors", [128, tot * 3], f16, kind="ExternalInput").ap()
    # 99 partition lines: 4 tile groups at partition offsets {0,32,64,96},
    # rows 3..31 of each group are don't-care (host reads rows 32*i..32*i+2)
    out_ext = nc.dram_tensor("out", [99, n_rounds * F], f16,
                             kind="ExternalOutput").ap()

    # per-round flat chunk lists: (slot, chunk-in-slot, global chunk idx)
    j = 0
    round_flat = []
    for r in range(n_rounds):
        fl = []
        for s in range(r * GROUP_TILES, (r + 1) * GROUP_TILES):
            for c in range(slot_nch[s]):
                fl.append((s, c, j))
                j += 1
        round_flat.append(fl)

    with tile.TileContext(nc) as tc:
        with (
            tc.tile_pool(name="consts", bufs=1) as consts,
            tc.tile_pool(name="work", bufs=3) as work,
            tc.tile_pool(name="outsb", bufs=1) as outsb,
            tc.tile_pool(name="psum_arg", bufs=2, space="PSUM") as psum_arg,
            tc.tile_pool(name="psum_out", bufs=2, space="PSUM") as psum_out,
        ):
            cb_sb = consts.tile([128, CB], f16, name="cb_sb")
            for q in range(QUADS):
                nc.sync.dma_start(
                    out=cb_sb[32 * q : 32 * q + K, :], in_=cb_ext[q]
                )
            colors_sb = consts.tile([128, tot * 3], f16, name="colors_sb")
            nc.sync.dma_start(out=colors_sb[:], in_=colors_ext[:])
            out_sb = outsb.tile([128, n_rounds * F], f16, name="out_sb")

            # preload the Exp activation table while the input DMAs are in
            # flight (the table load rides on this first tiny activation)
            warm_sb = consts.tile([1, 8], f16, name="warm_sb")
            nc.vector.memset(warm_sb[:], 0.0)
            nc.scalar.activation(
                warm_sb[:], warm_sb[:], mybir.ActivationFunctionType.Exp
            )

            for r in range(n_rounds):
                out_ps = psum_out.tile([99, F], f32, tag="out", name=f"out_ps_{r}")
                for rep in range(repeat):
                    fl = round_flat[r]
                    for g0 in range(0, len(fl), FUSE):
                        g = fl[g0 : g0 + FUSE]
                        glen = len(g)
                        arg_ps = psum_arg.tile([128, glen * F], f32, tag="arg")
                        for idx, (s, c, jj) in enumerate(g):
                            p0 = 32 * (jj % QUADS)
                            nc.tensor.matmul(
                                arg_ps[:, idx * F : (idx + 1) * F],
                                lhsT=cb_sb[p0 : p0 + K, jj * 128 : (jj + 1) * 128],
                                rhs=cb_sb[p0 : p0 + K, X : X + F],
                                start=True,
                                stop=True,
                                tile_position=(p0, 0),
                            )
                        alpha_sb = work.tile([128, glen * F], f16, tag="alpha")
                        nc.scalar.activation(
                            alpha_sb[:],
                            arg_ps[:],
                            mybir.ActivationFunctionType.Exp,
                        )
                        for idx, (s, c, jj) in enumerate(g):
                            i = s % GROUP_TILES
                            p0 = 32 * i
                            nc.tensor.matmul(
                                out_ps[p0 : p0 + 3, :],
                                lhsT=colors_sb[:, jj * 3 : (jj + 1) * 3],
                                rhs=alpha_sb[:, idx * F : (idx + 1) * F],
                                start=(c == 0 and rep == 0),
                                stop=(c == slot_nch[s] - 1 and rep == repeat - 1),
                                tile_position=(0, p0),
                            )
                nc.vector.tensor_copy(out_sb[0:99, r * F : (r + 1) * F], out_ps[:])
                nc.sync.dma_start(
                    out=out_ext[:, r * F : (r + 1) * F],
                    in_=out_sb[0:99, r * F : (r + 1) * F],
                )
    nc.compile()
    return nc


def _get_program(slot_nch, cutoff, repeat=1):
    key = (tuple(slot_nch), float(cutoff), repeat)
    if key not in _prog_cache:
        _prog_cache[key] = _build_program(slot_nch, cutoff, repeat)
    return _prog_cache[key]


def _basis():
    """[6, F] f64 basis terms in tile-local coords (all fp16-exact)."""
    ys = np.arange(TILE_ROWS, dtype=np.float64) + 0.5 - TILE_ROWS / 2
    xs = np.arange(TILE_COLS, dtype=np.float64) + 0.5 - TILE_COLS / 2
    yl = np.repeat(ys, TILE_COLS)
    xl = np.tile(xs, TILE_ROWS)
    return np.stack([xl * xl, xl * yl, yl * yl, xl, yl, np.ones_like(xl)], axis=0)


def kernel(
    opacity,
    means,
    stds,
    rhos,
    colors,
    image_height,
    image_width,
    scale_factor,
    raster_ratio,
    _repeat=1,
    _time_exec=False,
    _bench_calls=0,
):
    H = int(image_height)
    W = int(image_width)
    sf = float(scale_factor)
    rr = float(raster_ratio)
    opacity = np.asarray(opacity, np.float64)
    means = np.asarray(means, np.float64)
    stds = np.asarray(stds, np.float64) * sf
    rhos = np.asarray(rhos, np.float64)
    colors = np.asarray(colors, np.float32)
    N = opacity.shape[0]

    n_ty = H // TILE_ROWS
    n_tx = W // TILE_COLS
    n_tiles = n_ty * n_tx
    assert n_tiles % N_CORES == 0
    n_slots = n_tiles // N_CORES
    assert n_slots % GROUP_TILES == 0

    # --- per-gaussian inverse covariance (f64)
    sx, sy = stds[:, 0], stds[:, 1]
    om = 1.0 - rhos * rhos
    ia = 1.0 / (sx * sx * om)
    ib = -rhos / (sx * sy * om)
    ic = 1.0 / (sy * sy * om)
    mx, my = means[:, 0], means[:, 1]
    lnop = np.log(np.maximum(opacity, 1e-30))

    # --- exact ellipse-vs-rect cull: min Mahalanobis^2 over pixel centers
    cut2 = rr * rr + 1e-6

    def min_m2(x0, x1, y0, y1):
        dx0, dx1 = x0 - mx, x1 - mx
        dy0, dy1 = y0 - my, y1 - my
        inside = (dx0 <= 0) & (dx1 >= 0) & (dy0 <= 0) & (dy1 >= 0)
        best = np.full(N, np.inf)
        for cdx in (dx0, dx1):
            dy = np.clip(-ib * cdx / ic, dy0, dy1)
            best = np.minimum(best, ia * cdx * cdx + 2 * ib * cdx * dy + ic * dy * dy)
        for cdy in (dy0, dy1):
            dx = np.clip(-ib * cdy / ia, dx0, dx1)
            best = np.minimum(best, ia * dx * dx + 2 * ib * cdy * dx + ic * cdy * cdy)
        return np.where(inside, 0.0, best)

    tile_ids = []
    tile_pos = []
    for tyi in range(n_ty):
        ty = tyi * TILE_ROWS
        for txi in range(n_tx):
            tx = txi * TILE_COLS
            m2 = min_m2(tx + 0.5, tx + TILE_COLS - 0.5, ty + 0.5, ty + TILE_ROWS - 0.5)
            tile_ids.append(np.nonzero(m2 <= cut2)[0])
            tile_pos.append((ty, tx))

    # snake-deal tiles to cores by descending chunk need so the SPMD slot
    # capacities (max over cores per slot) hug each core's real need
    nchs = [max(1, (len(ids) + 127) // 128) for ids in tile_ids]
    t_order = sorted(range(n_tiles), key=lambda t: -nchs[t])
    assign = [[] for _ in range(N_CORES)]
    for i, t in enumerate(t_order):
        rnd, pos = divmod(i, N_CORES)
        core = pos if rnd % 2 == 0 else N_CORES - 1 - pos
        assign[core].append(t)
    # permute slots so round chunk-counts hug multiples of FUSE (heavy slots
    # paired with light slots within a round)
    perm = []
    lo, hi = 0, n_slots - 1
    while lo < hi:
        perm.extend([lo, lo + 1, hi - 1, hi])
        lo += 2
        hi -= 2
    assign = [[a[p] for p in perm] for a in assign]
    slot_nch = tuple(
        max(nchs[assign[core][k]] for core in range(N_CORES)) for k in range(n_slots)
    )
    tot = sum(slot_nch)
    X = tot * 128

    nc = _get_program(slot_nch, 0.0, _repeat)

    basis6 = _basis()  # [6, F] f64, fp16-exact values

    in_maps = []
    for core in range(N_CORES):
        cb_arr = np.zeros((QUADS, K, X + F), np.float16)
        colors_arr = np.zeros((128, tot * 3), np.float16)
        base = 0
        for k in range(n_slots):
            t = assign[core][k]
            ty, tx = tile_pos[t]
            ids = tile_ids[t]
            gn = len(ids)
            assert gn <= slot_nch[k] * 128
            if gn:
                cxo = tx + TILE_COLS / 2
                cyo = ty + TILE_ROWS / 2
                mxl = mx[ids] - cxo
                myl = my[ids] - cyo
                A, B, C = ia[ids], ib[ids], ic[ids]
                cf = np.stack(
                    [
                        -0.5 * A,
                        -B,
                        -0.5 * C,
                        A * mxl + B * myl,
                        B * mxl + C * myl,
                        -0.5 * (A * mxl * mxl + 2 * B * mxl * myl + C * myl * myl)
                        + lnop[ids],
                    ],
                    axis=0,
                )  # [6, gn] f64
                hi = cf.astype(np.float16)
                lo = (cf - hi.astype(np.float64)).astype(np.float16)
                col = colors[ids].astype(np.float16)
                for c in range((gn + 127) // 128):
                    lo_i, hi_i = c * 128, min((c + 1) * 128, gn)
                    jj = base + c
                    cb_arr[:, 0:6, jj * 128 : jj * 128 + hi_i - lo_i] = hi[:, lo_i:hi_i]
                    cb_arr[:, 6:12, jj * 128 : jj * 128 + hi_i - lo_i] = lo[:, lo_i:hi_i]
                    colors_arr[: hi_i - lo_i, jj * 3 : jj * 3 + 3] = col[lo_i:hi_i]
            base += slot_nch[k]
        cb_arr[:, 0:6, X : X + F] = basis6
        cb_arr[:, 6:12, X : X + F] = basis6
        in_maps.append({"cb": cb_arr, "colors": colors_arr})

    import time as _time

    global _last_in_maps
    _last_in_maps = in_maps
    run = _get_runner(nc)
    if _bench_calls:
        return run.time_loop(in_maps, _bench_calls)
    t0 = _time.time()
    results = run(in_maps, reuse_inputs=_time_exec)
    exec_wall = _time.time() - t0

    out = np.zeros((H, W, 3), np.float32)
    for core in range(N_CORES):
        o = results[core]["out"]  # [99, n_rounds*F] f16; rows 32i..32i+2 real
        for k in range(n_slots):
            ty, tx = tile_pos[assign[core][k]]
            r, i = divmod(k, GROUP_TILES)
            blk = o[32 * i : 32 * i + 3, r * F : (r + 1) * F].astype(
                np.float32
            ).reshape(3, TILE_ROWS, TILE_COLS)
            out[ty : ty + TILE_ROWS, tx : tx + TILE_COLS, :] = blk.transpose(1, 2, 0)
    if _repeat > 1:
        out /= np.float32(_repeat)
    if _time_exec:
        return out, exec_wall
    return out
